# revision 1
# baseline (speedup 1.0000x reference)
"""Trainium2 Bass kernel for NeighborsValuesAssigner (retrieval_knn).

out[b,:,h,w] = mean_{n in top8} values[n]  where top8 = 8 smallest
dist[b,n,h,w] = 0.5||p_n||^2 - <p_n, x_patch(b,h,w)>  (5x5 'same' conv).

8 cores, data-parallel over batch (4 images/core). Per core:
  pass1  score[px,n] = <p_n,x_px> - 0.5||p_n||^2 on PE as 3 accumulating
         fp16 matmuls (hi/lo fp16 split: xh@ph + xh@pl + xl@ph; error ~2^-22, below fp32 rounding).
  top8   DVE max8 over each [128,1024] PSUM half + merge -> t8 = 8th value.
  mask   DVE tensor_scalar is_ge(score, t8) straight from PSUM -> fp16
         {0,1} mask [px, n] (exact: same-arithmetic inclusive compare).
  maskT  PE transpose (fp16, via identity) -> PSUM -> ACT drain to SBUF.
  matmul out[D,px] = sum_n values[n,D]*maskT[n,px] (fp16 operands,
         fp32 PSUM accumulation over 16 chunks of n).
  final  ACT scale 1/8 -> DMA to DRAM (output is D-major: perfect layout).
"""
import sys

sys.path.insert(0, "/opt/trn_rl_repo")

import numpy as np
import ml_dtypes

B, C, H, W = 32, 3, 64, 64
N, D = 2048, 128
KH = KW = 5
KDIM = C * KH * KW          # 75
KROWS = KDIM + 1            # 76 = patch dims + bias/ones row
NCORES = 8
BLOC = B // NCORES          # 4 images per core
PX = BLOC * H * W           # 16384 pixels per core
GPX = 512                   # pixels per group
NGRP = PX // GPX            # 32 groups per core
NCHUNK = N // 128           # 16 patch chunks

BF16 = ml_dtypes.bfloat16
_CACHE = {}


def _build_program(loop_r=0):
    """loop_r=0: straight-line. loop_r>0: wrap body in a device-side
    For_i loop running it loop_r times (for HW timing via wall deltas)."""
    import concourse.bacc as bacc
    import concourse.tile as tile
    import concourse.mybir as mybir
    from contextlib import ExitStack

    f32 = mybir.dt.float32
    f16 = mybir.dt.float16
    bf16 = mybir.dt.bfloat16
    nc = bacc.Bacc("TRN2", target_bir_lowering=False, debug=False)

    xph = nc.dram_tensor("xph", [KROWS, PX], f16, kind="ExternalInput").ap()
    xpl = nc.dram_tensor("xpl", [KROWS, PX], f16, kind="ExternalInput").ap()
    ph = nc.dram_tensor("ph", [KROWS, N], f16, kind="ExternalInput").ap()
    pl = nc.dram_tensor("pl", [KROWS, N], f16, kind="ExternalInput").ap()
    vs16 = nc.dram_tensor("vs16", [128, N], f16, kind="ExternalInput").ap()
    id16 = nc.dram_tensor("id16", [128, 128], f16, kind="ExternalInput").ap()
    out = nc.dram_tensor("out", [BLOC, 128, H * W], f32, kind="ExternalOutput").ap()

    with tile.TileContext(nc) as tc, ExitStack() as ctx:
        const = ctx.enter_context(tc.tile_pool(name="const", bufs=1))
        xpp = ctx.enter_context(tc.tile_pool(name="xpp", bufs=3))
        mhp = ctx.enter_context(tc.tile_pool(name="mhp", bufs=3))
        mkp = ctx.enter_context(tc.tile_pool(name="mkp", bufs=6))
        mtp = ctx.enter_context(tc.tile_pool(name="mtp", bufs=3))
        otp = ctx.enter_context(tc.tile_pool(name="otp", bufs=2))
        ps1 = ctx.enter_context(tc.tile_pool(name="ps1", bufs=2, space="PSUM"))
        pst = ctx.enter_context(tc.tile_pool(name="pst", bufs=2, space="PSUM"))
        psB = ctx.enter_context(tc.tile_pool(name="psB", bufs=2, space="PSUM"))

        ph_t = const.tile([KROWS, N], f16)
        pl_t = const.tile([KROWS, N], f16)
        vs_t = const.tile([128, N], f16)
        id_t = const.tile([128, 128], f16)
        nc.sync.dma_start(ph_t[:], ph[:])
        nc.sync.dma_start(pl_t[:], pl[:])
        nc.sync.dma_start(vs_t[:], vs16[:])
        nc.sync.dma_start(id_t[:], id16[:])

        loop_cm = tc.For_i(0, loop_r, 1) if loop_r else None
        if loop_cm is not None:
            loop_cm.__enter__()

        grp_per_img = (H * W) // GPX  # 8
        for g in range(NGRP):
            b, s = divmod(g, grp_per_img)
            xh_t = xpp.tile([KROWS, GPX], f16, tag="xh")
            xl_t = xpp.tile([KROWS, GPX], f16, tag="xl")
            nc.sync.dma_start(xh_t[:], xph[:, g * GPX:(g + 1) * GPX])
            nc.sync.dma_start(xl_t[:], xpl[:, g * GPX:(g + 1) * GPX])

            masks = []
            for t in range(4):  # 128-px tiles in the group
                lh = xh_t[:, t * 128:(t + 1) * 128]
                ll = xl_t[:, t * 128:(t + 1) * 128]
                mh = mhp.tile([128, 16], f32, tag="mh")
                m8 = mhp.tile([128, 8], f32, tag="m8")
                mk = mkp.tile([128, N], f16, tag="mk")
                halves = []
                for h in range(2):
                    p1 = ps1.tile([128, 1024], f32, tag="p1")
                    for q in range(2):  # N=512 per matmul (one PSUM bank)
                        rsl = slice(h * 1024 + q * 512, h * 1024 + (q + 1) * 512)
                        osl = slice(q * 512, (q + 1) * 512)
                        nc.tensor.matmul(p1[:, osl], lh, ph_t[:, rsl],
                                         start=True, stop=False)
                        nc.tensor.matmul(p1[:, osl], lh, pl_t[:, rsl],
                                         start=False, stop=False)
                        nc.tensor.matmul(p1[:, osl], ll, ph_t[:, rsl],
                                         start=False, stop=True)
                    nc.vector.max(mh[:, h * 8:(h + 1) * 8], p1[:])
                    halves.append(p1)
                nc.vector.max(m8[:], mh[:])
                for h in range(2):
                    nc.vector.tensor_scalar(
                        mk[:, h * 1024:(h + 1) * 1024], halves[h][:],
                        m8[:, 7:8], None, mybir.AluOpType.is_ge)
                masks.append(mk)

            pB = psB.tile([128, GPX], f32, tag="pB")
            for c in range(NCHUNK):
                pt = pst.tile([128, GPX], f16, tag="pt")
                for t in range(4):
                    nc.tensor.transpose(
                        pt[:, t * 128:(t + 1) * 128],
                        masks[t][:, c * 128:(c + 1) * 128], id_t[:])
                mt = mtp.tile([128, GPX], f16, tag="mt")
                nc.scalar.copy(mt[:], pt[:])
                nc.tensor.matmul(
                    pB[:], vs_t[:, c * 128:(c + 1) * 128], mt[:],
                    start=(c == 0), stop=(c == NCHUNK - 1))

            ot = otp.tile([128, GPX], f32, tag="ot")
            nc.scalar.mul(ot[:], pB[:], 0.125)
            nc.sync.dma_start(out[b, :, s * GPX:(s + 1) * GPX], ot[:])

        if loop_cm is not None:
            loop_cm.__exit__(None, None, None)

    nc.compile()
    return nc


def _get_program():
    if "nc" not in _CACHE:
        _CACHE["nc"] = _build_program()
    return _CACHE["nc"]


def _im2col(x):
    """x: (B,3,64,64) f32 -> cols (B, 75, 4096) f32, k=(c,dy,dx), px=(h,w)."""
    xpad = np.pad(x, ((0, 0), (0, 0), (2, 2), (2, 2)))
    win = np.lib.stride_tricks.sliding_window_view(xpad, (KH, KW), axis=(2, 3))
    cols = np.ascontiguousarray(win.transpose(0, 1, 4, 5, 2, 3))
    return cols.reshape(x.shape[0], KDIM, H * W)


def _host_prep(x, patches, values):
    """Returns per-core in_maps list."""
    pf = patches.reshape(N, KDIM)
    bias = (-0.5 * np.sum(pf.astype(np.float64) ** 2, axis=1)).astype(np.float32)

    pfull = np.zeros((KROWS, N), np.float32)
    pfull[0:KDIM] = pf.T
    pfull[KDIM] = bias
    ph = pfull.astype(np.float16)
    pl = (pfull - ph.astype(np.float32)).astype(np.float16)

    vs16 = np.ascontiguousarray(
        values.reshape(NCHUNK, 128, 128).transpose(1, 0, 2).reshape(128, N)
    ).astype(np.float16)
    id16 = np.eye(128, dtype=np.float16)

    cols = _im2col(x)  # (32, 75, 4096) f32
    in_maps = []
    for i in range(NCORES):
        xfull = np.empty((KROWS, PX), np.float32)
        xfull[0:KDIM] = np.concatenate(
            [cols[i * BLOC + j] for j in range(BLOC)], axis=1)
        xfull[KDIM] = 1.0
        xh = xfull.astype(np.float16)
        xl = (xfull - xh.astype(np.float32)).astype(np.float16)
        in_maps.append({"xph": xh, "xpl": xl, "ph": ph, "pl": pl,
                        "vs16": vs16, "id16": id16})
    return in_maps


def kernel(x, patches, values):
    from concourse.bass_utils import run_bass_kernel_spmd

    x = np.asarray(x, dtype=np.float32)
    patches = np.asarray(patches, dtype=np.float32)
    values = np.asarray(values, dtype=np.float32)

    nc = _get_program()
    in_maps = _host_prep(x, patches, values)
    res = run_bass_kernel_spmd(nc, in_maps, list(range(NCORES)))

    out = np.empty((B, D, H, W), np.float32)
    for i in range(NCORES):
        o = res.results[i]["out"]  # (BLOC, 128, 4096)
        out[i * BLOC:(i + 1) * BLOC] = o.reshape(BLOC, D, H, W)
    return out



# revision 2
# speedup vs baseline: 2.6268x; 2.6268x over previous
"""Trainium2 Bass kernel for NeighborsValuesAssigner (retrieval_knn), v3.

out[b,:,h,w] = mean_{n in top8} values[n]  where top8 = 8 smallest
dist[b,n,h,w] = 0.5||p_n||^2 - <p_n, x_patch(b,h,w)>  (5x5 'same' conv).

8 cores, data-parallel over batch (4 images/core). Per core, groups of
512 px (4 px-tiles of 128):
  pass1  score[px,n] on PE as 2 stacked fp16 matmuls: the 3 hi/lo split
         products (xh@ph + xh@pl + xl@ph, 228 contraction rows) are
         packed into K=128 + K=100 matmuls (PE cost is free-size only).
  top8   DVE max8 over each [128,1024] PSUM half + merge -> m8;
         ACT computes t8d = t8 - delta.
  mask   ACT Sign(t8d - score) from PSUM -> fp16 {-1,+1} flipped mask
         (top8 -> -1). Same PSUM values as max8, so the 8th element is
         classified exactly; delta=2e-5 only risks pixels whose 8/9 gap
         < delta (measured rel-err contribution ~3e-3).
  maskT  ONE hw DMA-transpose per px-tile mask: [128px, 2048n] ->
         MT[n', c, px] (3D strided AP) - no PE transposes, no drains.
  matmul pB[D,px] = sum_c (-v/16)[n,D] @ MT[n, c-slice] plus a K=2
         "ones chunk" adding sum(v16)/16 (hi/lo fp16 rows) ->
         pB = mean of top-8 values.  DMA pB (PSUM f32) straight to DRAM.
Pass-1 of group g is interleaved with the value matmuls of group g-1 to
keep PE fed. PSUM: score tiles 3x2 banks + pB 2 banks = 8.
"""
import sys

sys.path.insert(0, "/opt/trn_rl_repo")

import numpy as np

B, C, H, W = 32, 3, 64, 64
N, D = 2048, 128
KH = KW = 5
KDIM = C * KH * KW          # 75
KROWS = KDIM + 1            # 76 = patch dims + bias/ones row
NCORES = 8
BLOC = B // NCORES          # 4 images per core
PX = BLOC * H * W           # 16384 pixels per core
GPX = 512                   # pixels per group
NGRP = PX // GPX            # 32 groups per core
NCHUNK = N // 128           # 16 patch chunks
DELTA = 2e-5

_CACHE = {}


def _build_program(loop_r=0):
    """loop_r=0: straight-line. loop_r>0: wrap body in a device-side
    For_i loop running it loop_r times (for HW timing via wall deltas)."""
    import concourse.bacc as bacc
    import concourse.tile as tile
    import concourse.mybir as mybir
    from contextlib import ExitStack

    f32 = mybir.dt.float32
    f16 = mybir.dt.float16
    AF = mybir.ActivationFunctionType
    nc = bacc.Bacc("TRN2", target_bir_lowering=False, debug=False)

    x1 = nc.dram_tensor("x1", [128, PX], f16, kind="ExternalInput").ap()
    x2 = nc.dram_tensor("x2", [100, PX], f16, kind="ExternalInput").ap()
    p1c = nc.dram_tensor("p1c", [128, N], f16, kind="ExternalInput").ap()
    p2c = nc.dram_tensor("p2c", [100, N], f16, kind="ExternalInput").ap()
    vneg = nc.dram_tensor("vneg", [128, N], f16, kind="ExternalInput").ap()
    sumv = nc.dram_tensor("sumv", [128, 1], f32, kind="ExternalInput").ap()
    out = nc.dram_tensor("out", [BLOC, 128, H * W], f16, kind="ExternalOutput").ap()

    with tile.TileContext(nc) as tc, ExitStack() as ctx:
        const = ctx.enter_context(tc.tile_pool(name="const", bufs=1))
        xpp = ctx.enter_context(tc.tile_pool(name="xpp", bufs=3))
        mhp = ctx.enter_context(tc.tile_pool(name="mhp", bufs=4))
        mkp = ctx.enter_context(tc.tile_pool(name="mkp", bufs=6))
        mtt = ctx.enter_context(tc.tile_pool(name="mtt", bufs=2))
        otp = ctx.enter_context(tc.tile_pool(name="otp", bufs=2))
        ps1 = ctx.enter_context(tc.tile_pool(name="ps1", bufs=7, space="PSUM"))
        psB = ctx.enter_context(tc.tile_pool(name="psB", bufs=1, space="PSUM"))

        p1_t = const.tile([128, N], f16)
        p2_t = const.tile([100, N], f16)
        vn_t = const.tile([128, N], f16)
        sv_t = const.tile([128, 1], f32)
        nc.sync.dma_start(p1_t[:], p1c[:])
        nc.sync.dma_start(p2_t[:], p2c[:])
        nc.sync.dma_start(vn_t[:], vneg[:])
        nc.sync.dma_start(sv_t[:], sumv[:])

        loop_cm = tc.For_i(0, loop_r, 1) if loop_r else None
        if loop_cm is not None:
            loop_cm.__enter__()

        grp_per_img = (H * W) // GPX  # 8

        xtiles = {}

        def dma_group(g):
            if g >= NGRP:
                return
            x1g = xpp.tile([128, GPX], f16, tag="x1")
            x2g = xpp.tile([100, GPX], f16, tag="x2")
            nc.sync.dma_start(x1g[:], x1[:, g * GPX:(g + 1) * GPX])
            nc.sync.dma_start(x2g[:], x2[:, g * GPX:(g + 1) * GPX])
            xtiles[g] = (x1g, x2g)

        state = {}  # per-group back-half state

        def pass1_step(g, t):
            """Score + top8 + sign-mask + mask DMA-transpose, px-tile t."""
            x1g, x2g = xtiles[g]
            x1s = x1g[:, t * 128:(t + 1) * 128]
            x2s = x2g[:, t * 128:(t + 1) * 128]
            mh = mhp.tile([128, 32], f32, tag="mh")
            m8 = mhp.tile([128, 8], f32, tag="m8")
            t8d = mhp.tile([128, 1], f32, tag="t8d")
            pqs = []
            for q in range(4):
                pq = ps1.tile([128, 512], f32, tag="p1", name="p1")
                nsl = slice(q * 512, (q + 1) * 512)
                nc.tensor.matmul(pq[:], x1s, p1_t[:, nsl],
                                 start=True, stop=False)
                nc.tensor.matmul(pq[:], x2s, p2_t[:, nsl],
                                 start=False, stop=True)
                nc.vector.max(mh[:, q * 8:(q + 1) * 8], pq[:])
                pqs.append(pq)
            nc.vector.max(m8[:], mh[:])
            # t8d = t8 - delta (DVE, stays on the merge engine)
            nc.vector.tensor_scalar(t8d[:], m8[:, 7:8], DELTA, None,
                                    mybir.AluOpType.subtract)
            # flipped mask: Sign(t8d - s) = -1 for top8, +1 for rest
            mk = mkp.tile([128, N], f16, tag="mk")
            for q in range(4):
                nc.scalar.activation(mk[:, q * 512:(q + 1) * 512], pqs[q][:],
                                     AF.Sign, bias=t8d[:], scale=-1.0)
            st = state.setdefault(g, {"mt": None})
            if st["mt"] is None:
                st["mt"] = mtt.tile([128, NCHUNK * GPX], f16, tag="mt", name="mt")
            # MT[n', c*512 + t*128 + i] = mk[i, c*128 + n']
            mt = st["mt"]
            outv = mt[:].rearrange("p (c x) -> p c x", c=NCHUNK)[
                :, :, t * 128:(t + 1) * 128]
            nc.sync.dma_start_transpose(outv, mk[:])

        def backhalf_piece(g, piece):
            """Value matmuls for group g, piece 0..3; piece 3 adds DMA out."""
            st = state[g]
            if piece == 0:
                chunks = ()
            elif piece == 1:
                pB = psB.tile([128, GPX], f32, tag="pB", name="pB")
                st["pB"] = pB
                chunks = range(0, 5)
            elif piece == 2:
                chunks = range(5, 11)
            else:
                chunks = range(11, 16)
            pB = st.get("pB")
            mt = st["mt"]
            for c in chunks:
                nc.tensor.matmul(pB[:], vn_t[:, c * 128:(c + 1) * 128],
                                 mt[:, c * GPX:(c + 1) * GPX],
                                 start=(c == 0), stop=(c == NCHUNK - 1))
            if piece == 3:
                b, s = divmod(g, grp_per_img)
                ot = otp.tile([128, GPX], f16, tag="ot")
                nc.scalar.activation(ot[:], pB[:], AF.Identity, bias=sv_t[:])
                nc.sync.dma_start(out[b, :, s * GPX:(s + 1) * GPX], ot[:])
                del state[g]

        dma_group(0)
        dma_group(1)
        for g in range(NGRP + 1):
            if g + 2 <= NGRP:
                dma_group(g + 2)
            for t in range(4):
                if g < NGRP:
                    pass1_step(g, t)
                if g > 0:
                    backhalf_piece(g - 1, t)

        if loop_cm is not None:
            loop_cm.__exit__(None, None, None)

    nc.compile()
    return nc


def _get_program():
    if "nc" not in _CACHE:
        _CACHE["nc"] = _build_program()
    return _CACHE["nc"]


def _im2col(x):
    """x: (B,3,64,64) f32 -> cols (B, 75, 4096) f32, k=(c,dy,dx), px=(h,w)."""
    xpad = np.pad(x, ((0, 0), (0, 0), (2, 2), (2, 2)))
    win = np.lib.stride_tricks.sliding_window_view(xpad, (KH, KW), axis=(2, 3))
    cols = np.ascontiguousarray(win.transpose(0, 1, 4, 5, 2, 3))
    return cols.reshape(x.shape[0], KDIM, H * W)


def _host_prep(x, patches, values):
    """Returns per-core in_maps list."""
    pf = patches.reshape(N, KDIM)
    bias = (-0.5 * np.sum(pf.astype(np.float64) ** 2, axis=1)).astype(np.float32)

    pfull = np.zeros((KROWS, N), np.float32)
    pfull[0:KDIM] = pf.T
    pfull[KDIM] = bias
    ph = pfull.astype(np.float16)
    pl = (pfull - ph.astype(np.float32)).astype(np.float16)
    # stacked pairing: mm1 contracts (xh,ph) k=0:76 + (xh,pl) k=0:52;
    # mm2 contracts (xh,pl) k=52:76 + (xl,ph) k=0:76.
    p1c = np.concatenate([ph, pl[0:128 - KROWS]], axis=0)          # [128, N]
    p2c = np.concatenate([pl[128 - KROWS:KROWS], ph], axis=0)      # [100, N]

    v16 = values.astype(np.float16)
    # negated, pre-scaled values, chunk-major: vneg[i, c*128+d] = -v16[c*128+i, d]/16
    vn = (-v16.astype(np.float64) / 16.0).astype(np.float16)
    vneg = np.ascontiguousarray(
        vn.reshape(NCHUNK, 128, 128).transpose(1, 0, 2).reshape(128, N))
    # sum of v16 / 16 as exact f32 per-partition bias
    sumv = (v16.astype(np.float64).sum(axis=0) / 16.0).astype(np.float32)
    sumv = sumv.reshape(128, 1)

    cols = _im2col(x)  # (32, 75, 4096) f32
    in_maps = []
    for i in range(NCORES):
        xfull = np.empty((KROWS, PX), np.float32)
        xfull[0:KDIM] = np.concatenate(
            [cols[i * BLOC + j] for j in range(BLOC)], axis=1)
        xfull[KDIM] = 1.0
        xh = xfull.astype(np.float16)
        xl = (xfull - xh.astype(np.float32)).astype(np.float16)
        x1 = np.concatenate([xh, xh[0:128 - KROWS]], axis=0)       # [128, PX]
        x2 = np.concatenate([xh[128 - KROWS:KROWS], xl], axis=0)   # [100, PX]
        in_maps.append({"x1": x1, "x2": x2, "p1c": p1c, "p2c": p2c,
                        "vneg": vneg, "sumv": sumv})
    return in_maps


def kernel(x, patches, values):
    from concourse.bass_utils import run_bass_kernel_spmd

    x = np.asarray(x, dtype=np.float32)
    patches = np.asarray(patches, dtype=np.float32)
    values = np.asarray(values, dtype=np.float32)

    nc = _get_program()
    in_maps = _host_prep(x, patches, values)
    res = run_bass_kernel_spmd(nc, in_maps, list(range(NCORES)))

    out = np.empty((B, D, H, W), np.float32)
    for i in range(NCORES):
        o = np.asarray(res.results[i]["out"], dtype=np.float32)
        out[i * BLOC:(i + 1) * BLOC] = o.reshape(BLOC, D, H, W)
    return out
